# revision 1
# baseline (speedup 1.0000x reference)
"""GAT (bipartite GATConv + mean-pool + 2 FC) on 8 Trainium2 NeuronCores.

Strategy: shard destination nodes across the 8 cores (each core owns N/8 dst
nodes and all edges pointing at them) so the segment softmax is fully local to
a core — no collectives needed.  Per core:

  Phase A: dense matmuls build a node table  row[n] = [h_s[n] (36) | a_s[n] (3)]
           in core-local DRAM (h_s = x_s @ W, a_s folded as x_s @ (W*att_src)),
           plus per-dst-node a_t values kept in SBUF.
  Phase B: dst nodes are processed in tiles of 128 (one node per partition,
           nodes degree-sorted so tiles have uniform run lengths).  Each node's
           incoming edges occupy L slots along its partition's free dimension;
           slot data arrives via indirect DMA row gathers from the table.
           The segment softmax (skipping the max-subtraction: logits are
           bounded, exp is safe in fp32) and the weighted message sum are
           strided DVE/ACT ops along each partition's run.  A one-hot matmul
           pools relu(out)·W2 and node counts into per-batch partials.

Host work is limited to index manipulation (edge sorting / padding / layout),
weight folding, and the final unsharding reduction of 8 x [128,2] partials.
"""

import numpy as np

import concourse.bacc as bacc
import concourse.tile as tile
from concourse import mybir
from concourse.bass import IndirectOffsetOnAxis
from concourse.bass_utils import run_bass_kernel_spmd

F32 = mybir.dt.float32
I32 = mybir.dt.int32

N_CORES = 8
P = 128
HEADS = 3
CH = 12
HC = HEADS * CH          # 36
ROW = HC + 4             # table row: 36 h | 3 a_s | 1 pad  (40 f32 = 160B)
PAD_A = -300.0           # pad-slot a_s value: exp(0.2*-300) = e^-60 ~ 0
NEG_SLOPE = 0.2

_nc_cache = {}


def _build_nc(in_dim, n_src_tiles, n_dst_tiles, L_list, slot_tot, n_xt_cols):
    """Build the SPMD Bass program (identical for all cores)."""
    key = (in_dim, n_src_tiles, n_dst_tiles, tuple(L_list), slot_tot, n_xt_cols)
    if key in _nc_cache:
        return _nc_cache[key]

    table_rows = n_src_tiles * P + 1          # +1 pad row
    pad_row_idx = n_src_tiles * P
    xs_cols = n_src_tiles * P
    half_tiles = n_src_tiles // 2             # n_src_tiles forced even by caller

    nc = bacc.Bacc("TRN2", target_bir_lowering=False, debug=False)
    d_xs = nc.dram_tensor("xs_t", [2 * in_dim, xs_cols // 2], F32, kind="ExternalInput")
    d_xt = nc.dram_tensor("xt_t", [in_dim, n_xt_cols], F32, kind="ExternalInput")
    d_idx = nc.dram_tensor("idxs", [P, slot_tot], I32, kind="ExternalInput")
    d_bc = nc.dram_tensor("bidcnt", [P, n_dst_tiles * 2], F32, kind="ExternalInput")
    d_wf = nc.dram_tensor("wfold", [in_dim, ROW], F32, kind="ExternalInput")
    d_wt = nc.dram_tensor("wat", [in_dim, 4], F32, kind="ExternalInput")
    d_w2 = nc.dram_tensor("w2b", [P, HC], F32, kind="ExternalInput")
    d_bb = nc.dram_tensor("biasb", [P, HC], F32, kind="ExternalInput")
    d_pr = nc.dram_tensor("padrow", [1, ROW], F32, kind="ExternalInput")
    d_q = nc.dram_tensor("q_out", [P, 2], F32, kind="ExternalOutput")

    with tile.TileContext(nc) as tc:
        with tc.tile_pool(name="const", bufs=1) as cpool, \
             tc.tile_pool(name="dram", bufs=1, space="DRAM") as dpool, \
             tc.tile_pool(name="xload", bufs=3) as xpool, \
             tc.tile_pool(name="tabout", bufs=4) as topool, \
             tc.tile_pool(name="gat", bufs=6) as gpool, \
             tc.tile_pool(name="work", bufs=3) as wpool, \
             tc.tile_pool(name="psA", bufs=4, space="PSUM") as psA, \
             tc.tile_pool(name="psB", bufs=2, space="PSUM") as psB, \
             tc.tile_pool(name="psT", bufs=2, space="PSUM") as psT:

            table = dpool.tile([table_rows, ROW], F32)

            # ---- constants into SBUF ----
            t_wf = cpool.tile([in_dim, ROW], F32)
            nc.sync.dma_start(t_wf[:], d_wf[:])
            t_wf2 = cpool.tile([2 * in_dim, ROW], F32)
            nc.sync.dma_start(t_wf2[0:in_dim, :], d_wf[:])
            nc.sync.dma_start(t_wf2[in_dim:2 * in_dim, :], d_wf[:])
            t_wt = cpool.tile([in_dim, 4], F32)
            nc.sync.dma_start(t_wt[:], d_wt[:])
            t_w2 = cpool.tile([P, HC], F32)
            nc.sync.dma_start(t_w2[:], d_w2[:])
            t_bb = cpool.tile([P, HC], F32)
            nc.sync.dma_start(t_bb[:], d_bb[:])
            t_pr = cpool.tile([1, ROW], F32)
            nc.sync.dma_start(t_pr[:], d_pr[:])
            t_idx = cpool.tile([P, slot_tot], I32)
            nc.sync.dma_start(t_idx[:], d_idx[:])
            t_bc = cpool.tile([P, n_dst_tiles * 2], F32)
            nc.sync.dma_start(t_bc[:], d_bc[:])
            t_xt = cpool.tile([in_dim, n_xt_cols], F32)
            nc.sync.dma_start(t_xt[:], d_xt[:])

            t_iota_i = cpool.tile([P, P], I32)
            nc.gpsimd.iota(t_iota_i[:], pattern=[[1, P]], base=0, channel_multiplier=0)
            t_iota = cpool.tile([P, P], F32)
            nc.vector.tensor_copy(t_iota[:], t_iota_i[:])

            t_qacc = cpool.tile([P, 2], F32)
            nc.vector.memset(t_qacc[:], 0.0)

            # ---- phase A2: a_t per dst tile -> resident SBUF ----
            t_at = cpool.tile([P, n_dst_tiles * 4], F32)
            for t in range(n_dst_tiles):
                ps = psT.tile([P, 4], F32, space="PSUM", tag="psat")
                nc.tensor.matmul(
                    ps[:], lhsT=t_xt[:, t * P:(t + 1) * P], rhs=t_wt[:],
                    start=True, stop=True)
                nc.scalar.copy(t_at[:, t * 4:(t + 1) * 4], ps[:])

            # ---- phase A: node table (h_s | a_s) ----
            # x packed [128, half]: partitions 0:64 = tiles [0, half), 64:128 =
            # tiles [half, 2*half). Two K=64 matmuls per slice in separate PE
            # row groups; 4 tiles batched per psum bank per half.
            XB = 8  # half-tiles per x-chunk load
            for blk in range(0, half_tiles, XB):
                nb = min(XB, half_tiles - blk)
                xs_sb = xpool.tile([2 * in_dim, XB * P], F32, tag="xs")
                nc.sync.dma_start(
                    xs_sb[:, : nb * P], d_xs[:, blk * P:(blk + nb) * P])
                for g in range(0, nb, 4):
                    ng = min(4, nb - g)
                    for hf in range(2):
                        ps = psA.tile([P, 4 * ROW], F32, space="PSUM", tag="psa")
                        for j in range(ng):
                            nc.tensor.matmul(
                                ps[:, j * ROW:(j + 1) * ROW],
                                lhsT=xs_sb[hf * in_dim:(hf + 1) * in_dim,
                                           (g + j) * P:(g + j + 1) * P],
                                rhs=t_wf2[hf * in_dim:(hf + 1) * in_dim, :],
                                start=True, stop=True)
                        ob = topool.tile([P, 4 * ROW], F32, tag="tab")
                        nc.vector.tensor_copy(ob[:, : ng * ROW], ps[:, : ng * ROW])
                        base = (hf * half_tiles + blk + g) * P
                        out_ap = table[:][base:base + ng * P, :]
                        out_ap = out_ap.rearrange("(j p) c -> p j c", p=P)
                        nc.scalar.dma_start(
                            out_ap,
                            ob[:, : ng * ROW].rearrange("p (j c) -> p j c", c=ROW))
            # pad row
            nc.scalar.dma_start(table[:][pad_row_idx:pad_row_idx + 1, :], t_pr[:])

            # ---- phase B ----
            off = 0
            for t in range(n_dst_tiles):
                L = L_list[t]
                g = gpool.tile([P, L * ROW], F32, tag="G")
                for s in range(L):
                    nc.gpsimd.indirect_dma_start(
                        out=g[:, s * ROW:(s + 1) * ROW],
                        out_offset=None,
                        in_=table[:],
                        in_offset=IndirectOffsetOnAxis(
                            ap=t_idx[:, off + s:off + s + 1], axis=0),
                    )
                off += L
                g3 = g[:].rearrange("p (l c) -> p l c", c=ROW)

                # logits l = a_s + a_t  (per head, a_t per-partition scalar)
                tT = wpool.tile([P, L * HEADS], F32, tag="T")
                T3 = tT[:].rearrange("p (l h) -> p l h", h=HEADS)
                for h in range(HEADS):
                    nc.vector.tensor_scalar_add(
                        T3[:, :, h], g3[:, :, HC + h], t_at[:, t * 4 + h:t * 4 + h + 1])
                # e = exp(leaky_relu(l))
                tE = wpool.tile([P, L * HEADS], F32, tag="E")
                nc.vector.tensor_scalar_mul(tE[:], tT[:], NEG_SLOPE)
                nc.vector.tensor_tensor(
                    out=tE[:], in0=tE[:], in1=tT[:], op=mybir.AluOpType.max)
                nc.scalar.activation(tE[:], tE[:], mybir.ActivationFunctionType.Exp)
                E3 = tE[:].rearrange("p (l h) -> p l h", h=HEADS)

                # denom + reciprocal
                t_den = wpool.tile([P, HEADS], F32, tag="den")
                nc.vector.tensor_reduce(
                    out=t_den[:], in_=E3.transpose([0, 2, 1]),
                    axis=mybir.AxisListType.X, op=mybir.AluOpType.add)
                nc.vector.tensor_scalar_max(t_den[:], t_den[:], 1e-30)
                t_rec = wpool.tile([P, HEADS], F32, tag="rec")
                nc.vector.reciprocal(t_rec[:], t_den[:])

                # weighted message sum U = sum_l e * h
                tM = wpool.tile([P, L * HC], F32, tag="M")
                M3 = tM[:].rearrange("p (l j) -> p l j", j=HC)
                e_b = E3.unsqueeze(3).to_broadcast((P, L, HEADS, CH))
                nc.vector.tensor_tensor(
                    out=M3[:], in0=g3[:, :, 0:HC], in1=e_b, op=mybir.AluOpType.mult)
                tU = wpool.tile([P, HC], F32, tag="U")
                nc.vector.tensor_reduce(
                    out=tU[:], in_=M3.transpose([0, 2, 1]),
                    axis=mybir.AxisListType.X, op=mybir.AluOpType.add)

                # out = relu(U / denom + bias)
                tV = wpool.tile([P, HC], F32, tag="V")
                rec_b = t_rec[:].unsqueeze(2).to_broadcast((P, HEADS, CH))
                nc.vector.tensor_tensor(
                    out=tV[:].rearrange("p (h c) -> p h c", c=CH),
                    in0=tU[:].rearrange("p (h c) -> p h c", c=CH),
                    in1=rec_b, op=mybir.AluOpType.mult)
                nc.vector.tensor_tensor(
                    out=tV[:], in0=tV[:], in1=t_bb[:], op=mybir.AluOpType.add)
                nc.scalar.activation(tV[:], tV[:], mybir.ActivationFunctionType.Relu)

                # rv = sum(V * W2); RV = [rv | cnt]
                tR = wpool.tile([P, HC], F32, tag="R")
                nc.vector.tensor_tensor(
                    out=tR[:], in0=tV[:], in1=t_w2[:], op=mybir.AluOpType.mult)
                tRV = wpool.tile([P, 2], F32, tag="RV")
                nc.vector.tensor_reduce(
                    out=tRV[:, 0:1], in_=tR[:], axis=mybir.AxisListType.X,
                    op=mybir.AluOpType.add)
                nc.scalar.copy(tRV[:, 1:2], t_bc[:, 2 * t + 1:2 * t + 2])

                # pool into batches: q += onehot(bid)^T @ RV
                t_oh = wpool.tile([P, P], F32, tag="oh")
                nc.vector.tensor_scalar(
                    out=t_oh[:], in0=t_iota[:], scalar1=t_bc[:, 2 * t:2 * t + 1],
                    scalar2=None, op0=mybir.AluOpType.is_equal)
                ps_q = psB.tile([P, 2], F32, space="PSUM", tag="q")
                nc.tensor.matmul(ps_q[:], lhsT=t_oh[:], rhs=tRV[:],
                                 start=True, stop=True)
                nc.vector.tensor_tensor(
                    out=t_qacc[:], in0=t_qacc[:], in1=ps_q[:],
                    op=mybir.AluOpType.add)

            nc.sync.dma_start(d_q[:], t_qacc[:])
    nc.finalize()
    _nc_cache[key] = nc
    return nc


def kernel(**inputs):
    x_s = np.asarray(inputs["x_s"], np.float32)
    x_t = np.asarray(inputs["x_t"], np.float32)
    edge_index = np.asarray(inputs["edge_index"])
    x_s_batch = np.asarray(inputs["x_s_batch"]).astype(np.int64)
    W = np.asarray(inputs["W"], np.float32)
    att_src = np.asarray(inputs["att_src"], np.float32)
    att_dst = np.asarray(inputs["att_dst"], np.float32)
    bias = np.asarray(inputs["bias"], np.float32)
    fc1_w = np.asarray(inputs["fc1_w"], np.float32)
    fc1_b = np.asarray(inputs["fc1_b"], np.float32)
    fc3_w = np.asarray(inputs["fc3_w"], np.float32)
    fc3_b = np.asarray(inputs["fc3_b"], np.float32)

    n_nodes, in_dim = x_s.shape
    src = edge_index[0].astype(np.int64)
    dst = edge_index[1].astype(np.int64)

    # ---- host: edge bucketing by destination (layout prep only) ----
    deg = np.bincount(dst, minlength=n_nodes)
    order = np.argsort(-deg, kind="stable")      # nodes by degree desc
    # round-robin deal over cores: core c gets ranks c, c+8, ...
    nodes_per_core = (n_nodes + N_CORES - 1) // N_CORES
    n_dst_tiles = (nodes_per_core + P - 1) // P
    # per-tile run length: max degree in the global rank band of the tile
    L_list = []
    for t in range(n_dst_tiles):
        r0 = t * P * N_CORES
        L_list.append(max(1, int(deg[order[min(r0, n_nodes - 1)]])))
    slot_tot = int(np.sum(L_list))

    n_src_tiles = (n_nodes + P - 1) // P
    if n_src_tiles % 2:
        n_src_tiles += 1
    pad_row_idx = n_src_tiles * P
    xs_cols = n_src_tiles * P
    n_xt_cols = n_dst_tiles * P

    # edges sorted by dst -> per-node contiguous src runs
    e_order = np.argsort(dst, kind="stable")
    src_sorted = src[e_order].astype(np.int32)
    starts = np.searchsorted(dst[e_order], np.arange(n_nodes))
    ends = np.searchsorted(dst[e_order], np.arange(n_nodes) + 1)

    xs_t = np.zeros((in_dim, xs_cols), np.float32)
    xs_t[:, :n_nodes] = x_s.T
    half_cols = xs_cols // 2
    xs_t = np.concatenate([xs_t[:, :half_cols], xs_t[:, half_cols:]], axis=0)
    xs_t = np.ascontiguousarray(xs_t)

    # fold weights (host weight prep)
    wa_t = np.einsum("khc,hc->kh", W.reshape(in_dim, HEADS, CH), att_dst).astype(np.float32)
    wa_s = np.einsum("khc,hc->kh", W.reshape(in_dim, HEADS, CH), att_src).astype(np.float32)
    wfold = np.zeros((in_dim, ROW), np.float32)
    wfold[:, :HC] = W
    wfold[:, HC:HC + HEADS] = wa_s
    wat = np.zeros((in_dim, 4), np.float32)
    wat[:, :HEADS] = wa_t
    w2 = (fc1_w @ fc3_w)[:, 0].astype(np.float32)      # [36]
    w2b = np.tile(w2[None, :], (P, 1))
    biasb = np.tile(bias[None, :], (P, 1))
    padrow = np.zeros((1, ROW), np.float32)
    padrow[0, HC:HC + HEADS] = PAD_A

    in_maps = []
    for c in range(N_CORES):
        node_ids = order[c::N_CORES]             # this core's dst nodes, deg-sorted
        ncnt = len(node_ids)
        idxs = np.full((P, slot_tot), pad_row_idx, np.int32)
        bidcnt = np.zeros((P, n_dst_tiles * 2), np.float32)
        xt_t = np.zeros((in_dim, n_xt_cols), np.float32)
        off = 0
        for t in range(n_dst_tiles):
            L = L_list[t]
            for i in range(P):
                k = t * P + i
                if k >= ncnt:
                    continue
                node = node_ids[k]
                s0, s1 = starts[node], ends[node]
                d = s1 - s0
                idxs[i, off:off + d] = src_sorted[s0:s1]
                bidcnt[i, 2 * t] = float(x_s_batch[node])
                bidcnt[i, 2 * t + 1] = 1.0
            off += L
        valid = min(ncnt, n_dst_tiles * P)
        xt_t[:, :valid] = x_t[node_ids[:valid]].T
        in_maps.append({
            "xs_t": xs_t, "xt_t": xt_t, "idxs": idxs, "bidcnt": bidcnt,
            "wfold": wfold, "wat": wat, "w2b": w2b, "biasb": biasb,
            "padrow": padrow,
        })

    nc = _build_nc(in_dim, n_src_tiles, n_dst_tiles, L_list, slot_tot, n_xt_cols)
    res = run_bass_kernel_spmd(nc, in_maps, core_ids=list(range(N_CORES)))

    q = np.zeros((P, 2), np.float64)
    for c in range(N_CORES):
        q += res.results[c]["q_out"]
    cnt = np.maximum(q[:, 1], 1.0)
    out = q[:, 0] / cnt
    const = float(fc1_b @ fc3_w[:, 0] + fc3_b[0])
    return (out + const).astype(np.float32)



# revision 3
# speedup vs baseline: 8.3969x; 8.3969x over previous
"""GAT (bipartite GATConv + mean-pool + 2 FC) on 8 Trainium2 NeuronCores.

Strategy: shard destination nodes across the 8 cores (each core owns N/8 dst
nodes and all edges pointing at them) so the segment softmax is fully local to
a core — no collectives needed.  The host stages each core's edge shard as a
dst-major packed copy of x_s rows (pure index manipulation); per core:

  Per dst tile of 128 nodes (one node per partition, degree-sorted so tiles
  have uniform run lengths L): K=128 block-diagonal bf16 matmuls compute
  [h_s | a_s] for two edge slots at a time, landing PSUM results directly in
  the [dst x slot] layout the segment ops need — no DRAM table, no indirect
  DMA.  The segment softmax (max-subtraction skipped: logits bounded, exp
  safe in fp32) and weighted message sum are DVE/ACT ops along each
  partition's run.  A one-hot matmul pools relu(out)*W2 and node counts into
  per-batch partials.

Pad slots use a host-solved x-vector v_pad with v_pad @ (W*att_src) = -300
per head, so padded edges vanish under exp just like a pad table row.

Host work is limited to index manipulation (edge sorting / padding / packed
layout), weight folding, and the final unsharding reduction of 8 x [128,2]
partials.
"""

import numpy as np
import ml_dtypes

import concourse.bacc as bacc
import concourse.tile as tile
from concourse import mybir
from concourse.bass_utils import run_bass_kernel_spmd

F32 = mybir.dt.float32
BF16 = mybir.dt.bfloat16

N_CORES = 8
P = 128
HEADS = 3
CH = 12
HC = HEADS * CH          # 36
ROW = HC + 4             # matmul out per slot: 36 h | 3 a_s | 1 pad
PAD_A = -300.0           # pad-slot a_s value: exp(0.2*-300) = e^-60 ~ 0
NEG_SLOPE = 0.2

_nc_cache = {}


def _build_nc(in_dim, n_dst_tiles, n_xt_dbl, PT_list):
    """Build the SPMD Bass program (identical for all cores)."""
    key = (in_dim, n_dst_tiles, n_xt_dbl, tuple(PT_list))
    if key in _nc_cache:
        return _nc_cache[key]

    PT_max = max(PT_list)
    xe_cols = sum(PT_list) * P

    nc = bacc.Bacc("TRN2", target_bir_lowering=False, debug=False)
    d_xe = nc.dram_tensor("xe", [2 * in_dim, xe_cols], BF16, kind="ExternalInput")
    d_xt = nc.dram_tensor("xt_p", [2 * in_dim, n_xt_dbl * P], BF16, kind="ExternalInput")
    d_bc = nc.dram_tensor("bidcnt", [P, n_dst_tiles * 2], F32, kind="ExternalInput")
    d_wf2 = nc.dram_tensor("wf2", [2 * in_dim, 2 * ROW], BF16, kind="ExternalInput")
    d_wat2 = nc.dram_tensor("wat2", [2 * in_dim, 8], BF16, kind="ExternalInput")
    d_w2 = nc.dram_tensor("w2b", [P, HC], F32, kind="ExternalInput")
    d_bb = nc.dram_tensor("biasb", [P, HC], F32, kind="ExternalInput")
    d_q = nc.dram_tensor("q_out", [P, 2], F32, kind="ExternalOutput")

    with tile.TileContext(nc) as tc:
        with tc.tile_pool(name="const", bufs=1) as cpool, \
             tc.tile_pool(name="xload", bufs=3) as xpool, \
             tc.tile_pool(name="gat", bufs=4) as gpool, \
             tc.tile_pool(name="work", bufs=3) as wpool, \
             tc.tile_pool(name="psA", bufs=3, space="PSUM") as psA, \
             tc.tile_pool(name="psB", bufs=2, space="PSUM") as psB, \
             tc.tile_pool(name="psT", bufs=2, space="PSUM") as psT:

            # ---- constants into SBUF ----
            t_wf2 = cpool.tile([2 * in_dim, 2 * ROW], BF16)
            nc.sync.dma_start(t_wf2[:], d_wf2[:])
            t_wat2 = cpool.tile([2 * in_dim, 8], BF16)
            nc.sync.dma_start(t_wat2[:], d_wat2[:])
            t_w2 = cpool.tile([P, HC], F32)
            nc.sync.dma_start(t_w2[:], d_w2[:])
            t_bb = cpool.tile([P, HC], F32)
            nc.sync.dma_start(t_bb[:], d_bb[:])
            t_bc = cpool.tile([P, n_dst_tiles * 2], F32)
            nc.sync.dma_start(t_bc[:], d_bc[:])
            t_xt = cpool.tile([2 * in_dim, n_xt_dbl * P], BF16)
            nc.sync.dma_start(t_xt[:], d_xt[:])

            t_iota_i = cpool.tile([P, P], mybir.dt.int32)
            nc.gpsimd.iota(t_iota_i[:], pattern=[[1, P]], base=0, channel_multiplier=0)
            t_iota = cpool.tile([P, P], F32)
            nc.vector.tensor_copy(t_iota[:], t_iota_i[:])

            t_qacc = cpool.tile([P, 2], F32)
            nc.vector.memset(t_qacc[:], 0.0)

            # ---- a_t per dst double-tile -> resident SBUF ----
            # t_at[:, d*8+(t%2)*4+h] = a_t of node tile t=2d+(t%2), head h
            t_at = cpool.tile([P, n_xt_dbl * 8], F32)
            for d in range(n_xt_dbl):
                ps = psT.tile([P, 8], F32, space="PSUM", tag="psat")
                nc.tensor.matmul(
                    ps[:], lhsT=t_xt[:, d * P:(d + 1) * P], rhs=t_wat2[:],
                    start=True, stop=True)
                nc.scalar.copy(t_at[:, d * 8:(d + 1) * 8], ps[:])

            # ---- main loop over dst tiles ----
            xoff = 0
            for t in range(n_dst_tiles):
                PT = PT_list[t]
                L = 2 * PT
                xe = xpool.tile([2 * in_dim, PT_max * P], BF16, tag="xe")
                nc.sync.dma_start(xe[:, : PT * P], d_xe[:, xoff:xoff + PT * P])
                xoff += PT * P

                # h|a per edge slot via block-diag matmuls (2 slots / matmul)
                g = gpool.tile([P, 2 * PT_max * ROW], BF16, tag="G")
                for b in range(0, PT, 6):
                    nb = min(6, PT - b)
                    ps = psA.tile([P, 6 * 2 * ROW], F32, space="PSUM", tag="psa")
                    for j in range(nb):
                        nc.tensor.matmul(
                            ps[:, j * 2 * ROW:(j + 1) * 2 * ROW],
                            lhsT=xe[:, (b + j) * P:(b + j + 1) * P],
                            rhs=t_wf2[:], start=True, stop=True)
                    nc.scalar.copy(
                        g[:, b * 2 * ROW:(b + nb) * 2 * ROW], ps[:, : nb * 2 * ROW])

                g3 = g[:, : L * ROW].rearrange("p (l c) -> p l c", c=ROW)
                atc = (t // 2) * 8 + (t % 2) * 4

                # logits l = a_s + a_t  (per head, a_t per-partition scalar)
                tT = wpool.tile([P, 2 * PT_max * HEADS], F32, tag="T")
                T3 = tT[:, : L * HEADS].rearrange("p (l h) -> p l h", h=HEADS)
                for h in range(HEADS):
                    nc.vector.tensor_scalar_add(
                        T3[:, :, h], g3[:, :, HC + h],
                        t_at[:, atc + h:atc + h + 1])
                # e = exp(leaky_relu(l))
                tE = wpool.tile([P, 2 * PT_max * HEADS], F32, tag="E")
                nc.vector.tensor_scalar_mul(
                    tE[:, : L * HEADS], tT[:, : L * HEADS], NEG_SLOPE)
                nc.vector.tensor_tensor(
                    out=tE[:, : L * HEADS], in0=tE[:, : L * HEADS],
                    in1=tT[:, : L * HEADS], op=mybir.AluOpType.max)
                tEb = wpool.tile([P, 2 * PT_max * HEADS], BF16, tag="Eb")
                nc.scalar.activation(
                    tEb[:, : L * HEADS], tE[:, : L * HEADS],
                    mybir.ActivationFunctionType.Exp)
                E3 = tEb[:, : L * HEADS].rearrange("p (l h) -> p l h", h=HEADS)

                # denom + reciprocal
                t_den = wpool.tile([P, HEADS], F32, tag="den")
                nc.vector.tensor_reduce(
                    out=t_den[:], in_=E3.transpose([0, 2, 1]),
                    axis=mybir.AxisListType.X, op=mybir.AluOpType.add)
                nc.vector.tensor_scalar_max(t_den[:], t_den[:], 1e-30)
                t_rec = wpool.tile([P, HEADS], F32, tag="rec")
                nc.vector.reciprocal(t_rec[:], t_den[:])

                # weighted message sum U = sum_l e * h
                tM = wpool.tile([P, 2 * PT_max * HC], BF16, tag="M")
                M3 = tM[:, : L * HC].rearrange("p (l j) -> p l j", j=HC)
                e_b = E3.unsqueeze(3).to_broadcast((P, L, HEADS, CH))
                nc.vector.tensor_tensor(
                    out=M3[:], in0=g3[:, :, 0:HC], in1=e_b, op=mybir.AluOpType.mult)
                tU = wpool.tile([P, HC], F32, tag="U")
                nc.vector.tensor_reduce(
                    out=tU[:], in_=M3.transpose([0, 2, 1]),
                    axis=mybir.AxisListType.X, op=mybir.AluOpType.add)

                # out = relu(U / denom + bias)
                tV = wpool.tile([P, HC], F32, tag="V")
                rec_b = t_rec[:].unsqueeze(2).to_broadcast((P, HEADS, CH))
                nc.vector.tensor_tensor(
                    out=tV[:].rearrange("p (h c) -> p h c", c=CH),
                    in0=tU[:].rearrange("p (h c) -> p h c", c=CH),
                    in1=rec_b, op=mybir.AluOpType.mult)
                nc.vector.tensor_tensor(
                    out=tV[:], in0=tV[:], in1=t_bb[:], op=mybir.AluOpType.add)
                nc.scalar.activation(tV[:], tV[:], mybir.ActivationFunctionType.Relu)

                # rv = sum(V * W2); RV = [rv | cnt]
                tR = wpool.tile([P, HC], F32, tag="R")
                nc.vector.tensor_tensor(
                    out=tR[:], in0=tV[:], in1=t_w2[:], op=mybir.AluOpType.mult)
                tRV = wpool.tile([P, 2], F32, tag="RV")
                nc.vector.tensor_reduce(
                    out=tRV[:, 0:1], in_=tR[:], axis=mybir.AxisListType.X,
                    op=mybir.AluOpType.add)
                nc.scalar.copy(tRV[:, 1:2], t_bc[:, 2 * t + 1:2 * t + 2])

                # pool into batches: q += onehot(bid)^T @ RV
                t_oh = wpool.tile([P, P], F32, tag="oh")
                nc.vector.tensor_scalar(
                    out=t_oh[:], in0=t_iota[:], scalar1=t_bc[:, 2 * t:2 * t + 1],
                    scalar2=None, op0=mybir.AluOpType.is_equal)
                ps_q = psB.tile([P, 2], F32, space="PSUM", tag="q")
                nc.tensor.matmul(ps_q[:], lhsT=t_oh[:], rhs=tRV[:],
                                 start=True, stop=True)
                nc.vector.tensor_tensor(
                    out=t_qacc[:], in0=t_qacc[:], in1=ps_q[:],
                    op=mybir.AluOpType.add)

            nc.sync.dma_start(d_q[:], t_qacc[:])
    nc.finalize()
    _nc_cache[key] = nc
    return nc


def _pack_dbl(x, n_dbl, in_dim):
    """Pack [n_dbl*256, in_dim] node-major features into the K=128
    block-diagonal lhsT layout [2*in_dim, n_dbl*128] (bf16)."""
    a = x.reshape(n_dbl, 2, P, in_dim)
    return np.ascontiguousarray(
        a.transpose(1, 3, 0, 2).reshape(2 * in_dim, n_dbl * P)
    ).astype(ml_dtypes.bfloat16)


def kernel(**inputs):
    x_s = np.asarray(inputs["x_s"], np.float32)
    x_t = np.asarray(inputs["x_t"], np.float32)
    edge_index = np.asarray(inputs["edge_index"])
    x_s_batch = np.asarray(inputs["x_s_batch"]).astype(np.int64)
    W = np.asarray(inputs["W"], np.float32)
    att_src = np.asarray(inputs["att_src"], np.float32)
    att_dst = np.asarray(inputs["att_dst"], np.float32)
    bias = np.asarray(inputs["bias"], np.float32)
    fc1_w = np.asarray(inputs["fc1_w"], np.float32)
    fc1_b = np.asarray(inputs["fc1_b"], np.float32)
    fc3_w = np.asarray(inputs["fc3_w"], np.float32)
    fc3_b = np.asarray(inputs["fc3_b"], np.float32)

    n_nodes, in_dim = x_s.shape
    src = edge_index[0].astype(np.int64)
    dst = edge_index[1].astype(np.int64)

    # ---- host: edge bucketing by destination (layout prep only) ----
    deg = np.bincount(dst, minlength=n_nodes)
    order = np.argsort(-deg, kind="stable")      # nodes by degree desc
    nodes_per_core = (n_nodes + N_CORES - 1) // N_CORES
    n_dst_tiles = (nodes_per_core + P - 1) // P
    L_list = []
    for t in range(n_dst_tiles):
        r0 = t * P * N_CORES
        Lt = max(1, int(deg[order[min(r0, n_nodes - 1)]]))
        L_list.append(Lt + (Lt & 1))             # force even (2 slots/matmul)
    PT_list = [Lt // 2 for Lt in L_list]
    n_xt_dbl = (n_dst_tiles + 1) // 2

    # edges sorted by dst -> per-node contiguous src runs
    e_order = np.argsort(dst, kind="stable")
    src_sorted = src[e_order].astype(np.int64)
    starts = np.searchsorted(dst[e_order], np.arange(n_nodes))

    # fold weights (host weight prep)
    wa_t = np.einsum("khc,hc->kh", W.reshape(in_dim, HEADS, CH), att_dst).astype(np.float32)
    wa_s = np.einsum("khc,hc->kh", W.reshape(in_dim, HEADS, CH), att_src).astype(np.float32)
    wfold = np.zeros((in_dim, ROW), np.float32)
    wfold[:, :HC] = W
    wfold[:, HC:HC + HEADS] = wa_s
    wf2 = np.zeros((2 * in_dim, 2 * ROW), np.float32)
    wf2[:in_dim, :ROW] = wfold
    wf2[in_dim:, ROW:] = wfold
    wf2 = wf2.astype(ml_dtypes.bfloat16)
    wat4 = np.zeros((in_dim, 4), np.float32)
    wat4[:, :HEADS] = wa_t
    wat2 = np.zeros((2 * in_dim, 8), np.float32)
    wat2[:in_dim, :4] = wat4
    wat2[in_dim:, 4:] = wat4
    wat2 = wat2.astype(ml_dtypes.bfloat16)
    w2 = (fc1_w @ fc3_w)[:, 0].astype(np.float32)      # [36]
    w2b = np.tile(w2[None, :], (P, 1))
    biasb = np.tile(bias[None, :], (P, 1))

    # pad-slot x vector: v_pad @ wa_s = PAD_A for every head
    v_pad = wa_s @ np.linalg.solve(
        wa_s.T @ wa_s, np.full((HEADS,), PAD_A, np.float64)).astype(np.float32)
    x_ext = np.vstack([x_s, v_pad[None, :]])           # row n_nodes = pad

    slot_ar = {}
    for Lt in set(L_list):
        slot_ar[Lt] = np.arange(Lt)[None, :]

    in_maps = []
    for c in range(N_CORES):
        node_ids = order[c::N_CORES]             # this core's dst nodes, deg-sorted
        ncnt = len(node_ids)
        pad_nodes = n_dst_tiles * P - ncnt
        nodes_pad = np.concatenate(
            [node_ids, np.zeros(pad_nodes, np.int64)]) if pad_nodes else node_ids
        valid_row = np.arange(n_dst_tiles * P) < ncnt

        bidcnt = np.zeros((P, n_dst_tiles * 2), np.float32)
        xe_blocks = []
        for t in range(n_dst_tiles):
            Lt = L_list[t]
            nt = nodes_pad[t * P:(t + 1) * P]
            vr = valid_row[t * P:(t + 1) * P]
            lens = np.where(vr, deg[nt], 0)
            mask = slot_ar[Lt] < lens[:, None]           # [P, Lt]
            idt = np.full((P, Lt), n_nodes, np.int64)
            gather_pos = (starts[nt][:, None] + slot_ar[Lt])[mask]
            idt[mask] = src_sorted[gather_pos]
            Et = x_ext[idt]                              # [P, Lt, in_dim]
            Et = Et.reshape(P, Lt // 2, 2, in_dim).transpose(2, 3, 1, 0)
            xe_blocks.append(np.ascontiguousarray(
                Et.reshape(2 * in_dim, (Lt // 2) * P)))
            bidcnt[:, 2 * t] = np.where(vr, x_s_batch[nt], 999.0)
            bidcnt[:, 2 * t + 1] = vr.astype(np.float32)
        xe = np.concatenate(xe_blocks, axis=1).astype(ml_dtypes.bfloat16)

        valid = min(ncnt, n_dst_tiles * P)
        xt_pad = np.zeros((n_xt_dbl * 2 * P, in_dim), np.float32)
        xt_pad[:valid] = x_t[node_ids[:valid]]
        xt_p = _pack_dbl(xt_pad, n_xt_dbl, in_dim)
        in_maps.append({
            "xe": xe, "xt_p": xt_p, "bidcnt": bidcnt,
            "wf2": wf2, "wat2": wat2, "w2b": w2b, "biasb": biasb,
        })

    nc = _build_nc(in_dim, n_dst_tiles, n_xt_dbl, PT_list)
    res = run_bass_kernel_spmd(nc, in_maps, core_ids=list(range(N_CORES)))

    q = np.zeros((P, 2), np.float64)
    for c in range(N_CORES):
        q += res.results[c]["q_out"]
    cnt = np.maximum(q[:, 1], 1.0)
    out = q[:, 0] / cnt
    const = float(fc1_b @ fc3_w[:, 0] + fc3_b[0])
    return (out + const).astype(np.float32)


# revision 6
# speedup vs baseline: 8.9539x; 1.0663x over previous
"""GAT (bipartite GATConv + mean-pool + 2 FC) on 8 Trainium2 NeuronCores.

Strategy: shard destination nodes across the 8 cores (each core owns N/8 dst
nodes and all edges pointing at them) so the segment softmax is fully local to
a core — no collectives needed.  The host stages each core's edge shard as a
dst-major packed copy of x_s rows (pure index manipulation); per core:

  Per dst tile of 128 nodes (one node per partition, degree-sorted so tiles
  have uniform run lengths L): K=128 block-diagonal bf16 matmuls compute
  [h_s | a_s] for two edge slots at a time, landing PSUM results directly in
  the [dst x slot] layout the segment ops need — no DRAM table, no indirect
  DMA.  The segment softmax (max-subtraction skipped: logits bounded, exp
  safe in fp32) and weighted message sum are DVE/ACT ops along each
  partition's run.  A one-hot matmul pools relu(out)*W2 and node counts into
  per-batch partials.

Pad slots use a host-solved x-vector v_pad with v_pad @ (W*att_src) = -300
per head, so padded edges vanish under exp just like a pad table row.

Host work is limited to index manipulation (edge sorting / padding / packed
layout), weight folding, and the final unsharding reduction of 8 x [128,2]
partials.
"""

import numpy as np
import ml_dtypes

import concourse.bacc as bacc
import concourse.tile as tile
from concourse import mybir
from concourse.bass_utils import run_bass_kernel_spmd

F32 = mybir.dt.float32
BF16 = mybir.dt.bfloat16

N_CORES = 8
P = 128
HEADS = 3
CH = 12
HC = HEADS * CH          # 36
ROW = HC + 4             # matmul out per slot: 36 h | 3 a_s | 1 pad
PAD_A = -300.0           # pad-slot a_s value: exp(0.2*-300) = e^-60 ~ 0
NEG_SLOPE = 0.2

_nc_cache = {}


def _build_nc(in_dim, n_dst_tiles, n_xt_dbl, PT_list):
    """Build the SPMD Bass program (identical for all cores)."""
    key = (in_dim, n_dst_tiles, n_xt_dbl, tuple(PT_list))
    if key in _nc_cache:
        return _nc_cache[key]

    PT_max = max(PT_list)
    xe_cols = sum(PT_list) * P

    nc = bacc.Bacc("TRN2", target_bir_lowering=False, debug=False)
    d_xe = nc.dram_tensor("xe", [2 * in_dim, xe_cols], BF16, kind="ExternalInput")
    d_xt = nc.dram_tensor("xt_p", [2 * in_dim, n_xt_dbl * P], BF16, kind="ExternalInput")
    d_bc = nc.dram_tensor("bidcnt", [P, n_dst_tiles * 2], F32, kind="ExternalInput")
    d_wf2 = nc.dram_tensor("wf2", [2 * in_dim, 2 * ROW], BF16, kind="ExternalInput")
    d_wat2 = nc.dram_tensor("wat2", [2 * in_dim, 8], BF16, kind="ExternalInput")
    d_w2 = nc.dram_tensor("w2b", [P, HC], F32, kind="ExternalInput")
    d_bb = nc.dram_tensor("biasb", [P, HC], F32, kind="ExternalInput")
    d_q = nc.dram_tensor("q_out", [P, 2], F32, kind="ExternalOutput")

    with tile.TileContext(nc) as tc:
        with tc.tile_pool(name="const", bufs=1) as cpool, \
             tc.tile_pool(name="xload", bufs=3) as xpool, \
             tc.tile_pool(name="gat", bufs=4) as gpool, \
             tc.tile_pool(name="work", bufs=3) as wpool, \
             tc.tile_pool(name="psA", bufs=3, space="PSUM") as psA, \
             tc.tile_pool(name="psB", bufs=2, space="PSUM") as psB, \
             tc.tile_pool(name="psT", bufs=2, space="PSUM") as psT:

            # ---- constants into SBUF ----
            t_wf2 = cpool.tile([2 * in_dim, 2 * ROW], BF16)
            nc.sync.dma_start(t_wf2[:], d_wf2[:])
            t_wat2 = cpool.tile([2 * in_dim, 8], BF16)
            nc.sync.dma_start(t_wat2[:], d_wat2[:])
            t_w2 = cpool.tile([P, HC], F32)
            nc.sync.dma_start(t_w2[:], d_w2[:])
            t_bb = cpool.tile([P, HC], F32)
            nc.sync.dma_start(t_bb[:], d_bb[:])
            t_bc = cpool.tile([P, n_dst_tiles * 2], F32)
            nc.sync.dma_start(t_bc[:], d_bc[:])
            t_xt = cpool.tile([2 * in_dim, n_xt_dbl * P], BF16)
            nc.sync.dma_start(t_xt[:], d_xt[:])

            t_iota_i = cpool.tile([P, P], mybir.dt.int32)
            nc.gpsimd.iota(t_iota_i[:], pattern=[[1, P]], base=0, channel_multiplier=0)
            t_iota = cpool.tile([P, P], F32)
            nc.vector.tensor_copy(t_iota[:], t_iota_i[:])

            t_qacc = cpool.tile([P, 2], F32)
            nc.vector.memset(t_qacc[:], 0.0)

            # ---- a_t per dst double-tile -> resident SBUF ----
            # t_at[:, d*8+(t%2)*4+h] = a_t of node tile t=2d+(t%2), head h
            t_at = cpool.tile([P, n_xt_dbl * 8], F32)
            for d in range(n_xt_dbl):
                ps = psT.tile([P, 8], F32, space="PSUM", tag="psat")
                nc.tensor.matmul(
                    ps[:], lhsT=t_xt[:, d * P:(d + 1) * P], rhs=t_wat2[:],
                    start=True, stop=True)
                nc.scalar.copy(t_at[:, d * 8:(d + 1) * 8], ps[:])

            # ---- main loop over dst tiles ----
            xoff = 0
            for t in range(n_dst_tiles):
                PT = PT_list[t]
                L = 2 * PT
                xe = xpool.tile([2 * in_dim, PT_max * P], BF16, tag="xe")
                nc.sync.dma_start(xe[:, : PT * P], d_xe[:, xoff:xoff + PT * P])
                xoff += PT * P

                # h|a per edge slot via block-diag matmuls (2 slots / matmul)
                g = gpool.tile([P, 2 * PT_max * ROW], BF16, tag="G")
                for b in range(0, PT, 6):
                    nb = min(6, PT - b)
                    ps = psA.tile([P, 6 * 2 * ROW], F32, space="PSUM", tag="psa")
                    for j in range(nb):
                        nc.tensor.matmul(
                            ps[:, j * 2 * ROW:(j + 1) * 2 * ROW],
                            lhsT=xe[:, (b + j) * P:(b + j + 1) * P],
                            rhs=t_wf2[:], start=True, stop=True)
                    nc.scalar.copy(
                        g[:, b * 2 * ROW:(b + nb) * 2 * ROW], ps[:, : nb * 2 * ROW])

                g3 = g[:, : L * ROW].rearrange("p (l c) -> p l c", c=ROW)
                atc = (t // 2) * 8 + (t % 2) * 4

                # logits (h-major), leaky_relu fused as (T*0.2) max T,
                # e = exp with fused per-head denominator accumulation
                tT = wpool.tile([P, 2 * PT_max * HEADS], F32, tag="T")
                tLR = wpool.tile([P, 2 * PT_max * HEADS], F32, tag="LR")
                tEb = wpool.tile([P, 2 * PT_max * HEADS], BF16, tag="Eb")
                t_den = wpool.tile([P, HEADS], F32, tag="den")
                for h in range(HEADS):
                    nc.vector.tensor_scalar_add(
                        tT[:, h * L:(h + 1) * L], g3[:, :, HC + h],
                        t_at[:, atc + h:atc + h + 1])
                nc.vector.scalar_tensor_tensor(
                    out=tLR[:, : L * HEADS], in0=tT[:, : L * HEADS],
                    scalar=NEG_SLOPE, in1=tT[:, : L * HEADS],
                    op0=mybir.AluOpType.mult, op1=mybir.AluOpType.max)
                for h in range(HEADS):
                    nc.scalar.activation(
                        tEb[:, h * L:(h + 1) * L], tLR[:, h * L:(h + 1) * L],
                        mybir.ActivationFunctionType.Exp,
                        accum_out=t_den[:, h:h + 1])
                E3h = tEb[:, : L * HEADS].rearrange("p (h l) -> p h l", h=HEADS)

                nc.vector.tensor_scalar_max(t_den[:], t_den[:], 1e-30)
                t_rec = wpool.tile([P, HEADS], F32, tag="rec")
                nc.vector.reciprocal(t_rec[:], t_den[:])

                # weighted message sum U = sum_l e * h
                tM = wpool.tile([P, 2 * PT_max * HC], BF16, tag="M")
                M3 = tM[:, : L * HC].rearrange("p (l j) -> p l j", j=HC)
                e_b = E3h.transpose([0, 2, 1]).unsqueeze(3).to_broadcast(
                    (P, L, HEADS, CH))
                nc.vector.tensor_tensor(
                    out=M3[:], in0=g3[:, :, 0:HC], in1=e_b, op=mybir.AluOpType.mult)
                tU = wpool.tile([P, HC], F32, tag="U")
                nc.vector.tensor_reduce(
                    out=tU[:], in_=M3.transpose([0, 2, 1]),
                    axis=mybir.AxisListType.X, op=mybir.AluOpType.add)

                # out = relu(U / denom + bias)
                tV = wpool.tile([P, HC], F32, tag="V")
                rec_b = t_rec[:].unsqueeze(2).to_broadcast((P, HEADS, CH))
                nc.vector.tensor_tensor(
                    out=tV[:].rearrange("p (h c) -> p h c", c=CH),
                    in0=tU[:].rearrange("p (h c) -> p h c", c=CH),
                    in1=rec_b, op=mybir.AluOpType.mult)
                nc.vector.tensor_tensor(
                    out=tV[:], in0=tV[:], in1=t_bb[:], op=mybir.AluOpType.add)
                nc.scalar.activation(tV[:], tV[:], mybir.ActivationFunctionType.Relu)

                # rv = sum(V * W2) fused; RV = [rv | cnt]
                tR = wpool.tile([P, HC], F32, tag="R")
                tRV = wpool.tile([P, 2], F32, tag="RV")
                nc.vector.scalar_tensor_tensor(
                    out=tR[:], in0=tV[:], scalar=1.0, in1=t_w2[:],
                    op0=mybir.AluOpType.mult, op1=mybir.AluOpType.mult,
                    accum_out=tRV[:, 0:1])
                nc.scalar.copy(tRV[:, 1:2], t_bc[:, 2 * t + 1:2 * t + 2])

                # pool into batches: q += onehot(bid)^T @ RV
                t_oh = wpool.tile([P, P], F32, tag="oh")
                nc.vector.tensor_scalar(
                    out=t_oh[:], in0=t_iota[:], scalar1=t_bc[:, 2 * t:2 * t + 1],
                    scalar2=None, op0=mybir.AluOpType.is_equal)
                ps_q = psB.tile([P, 2], F32, space="PSUM", tag="q")
                nc.tensor.matmul(ps_q[:], lhsT=t_oh[:], rhs=tRV[:],
                                 start=True, stop=True)
                nc.vector.tensor_tensor(
                    out=t_qacc[:], in0=t_qacc[:], in1=ps_q[:],
                    op=mybir.AluOpType.add)

            nc.sync.dma_start(d_q[:], t_qacc[:])
    nc.finalize()
    _nc_cache[key] = nc
    return nc


def _pack_dbl(x, n_dbl, in_dim):
    """Pack [n_dbl*256, in_dim] node-major features into the K=128
    block-diagonal lhsT layout [2*in_dim, n_dbl*128] (bf16)."""
    a = x.reshape(n_dbl, 2, P, in_dim)
    return np.ascontiguousarray(
        a.transpose(1, 3, 0, 2).reshape(2 * in_dim, n_dbl * P)
    ).astype(ml_dtypes.bfloat16)


def kernel(**inputs):
    x_s = np.asarray(inputs["x_s"], np.float32)
    x_t = np.asarray(inputs["x_t"], np.float32)
    edge_index = np.asarray(inputs["edge_index"])
    x_s_batch = np.asarray(inputs["x_s_batch"]).astype(np.int64)
    W = np.asarray(inputs["W"], np.float32)
    att_src = np.asarray(inputs["att_src"], np.float32)
    att_dst = np.asarray(inputs["att_dst"], np.float32)
    bias = np.asarray(inputs["bias"], np.float32)
    fc1_w = np.asarray(inputs["fc1_w"], np.float32)
    fc1_b = np.asarray(inputs["fc1_b"], np.float32)
    fc3_w = np.asarray(inputs["fc3_w"], np.float32)
    fc3_b = np.asarray(inputs["fc3_b"], np.float32)

    n_nodes, in_dim = x_s.shape
    src = edge_index[0].astype(np.int64)
    dst = edge_index[1].astype(np.int64)

    # ---- host: edge bucketing by destination (layout prep only) ----
    deg = np.bincount(dst, minlength=n_nodes)
    order = np.argsort(-deg, kind="stable")      # nodes by degree desc
    nodes_per_core = (n_nodes + N_CORES - 1) // N_CORES
    n_dst_tiles = (nodes_per_core + P - 1) // P
    L_list = []
    for t in range(n_dst_tiles):
        r0 = t * P * N_CORES
        Lt = max(1, int(deg[order[min(r0, n_nodes - 1)]]))
        L_list.append(Lt + (Lt & 1))             # force even (2 slots/matmul)
    PT_list = [Lt // 2 for Lt in L_list]
    n_xt_dbl = (n_dst_tiles + 1) // 2

    # edges sorted by dst -> per-node contiguous src runs
    e_order = np.argsort(dst, kind="stable")
    src_sorted = src[e_order].astype(np.int64)
    starts = np.searchsorted(dst[e_order], np.arange(n_nodes))

    # fold weights (host weight prep)
    wa_t = np.einsum("khc,hc->kh", W.reshape(in_dim, HEADS, CH), att_dst).astype(np.float32)
    wa_s = np.einsum("khc,hc->kh", W.reshape(in_dim, HEADS, CH), att_src).astype(np.float32)
    wfold = np.zeros((in_dim, ROW), np.float32)
    wfold[:, :HC] = W
    wfold[:, HC:HC + HEADS] = wa_s
    wf2 = np.zeros((2 * in_dim, 2 * ROW), np.float32)
    wf2[:in_dim, :ROW] = wfold
    wf2[in_dim:, ROW:] = wfold
    wf2 = wf2.astype(ml_dtypes.bfloat16)
    wat4 = np.zeros((in_dim, 4), np.float32)
    wat4[:, :HEADS] = wa_t
    wat2 = np.zeros((2 * in_dim, 8), np.float32)
    wat2[:in_dim, :4] = wat4
    wat2[in_dim:, 4:] = wat4
    wat2 = wat2.astype(ml_dtypes.bfloat16)
    w2 = (fc1_w @ fc3_w)[:, 0].astype(np.float32)      # [36]
    w2b = np.tile(w2[None, :], (P, 1))
    biasb = np.tile(bias[None, :], (P, 1))

    # pad-slot x vector: v_pad @ wa_s = PAD_A for every head
    v_pad = wa_s @ np.linalg.solve(
        wa_s.T @ wa_s, np.full((HEADS,), PAD_A, np.float64)).astype(np.float32)
    x_ext = np.vstack([x_s, v_pad[None, :]])           # row n_nodes = pad

    slot_ar = {}
    for Lt in set(L_list):
        slot_ar[Lt] = np.arange(Lt)[None, :]

    in_maps = []
    for c in range(N_CORES):
        node_ids = order[c::N_CORES]             # this core's dst nodes, deg-sorted
        ncnt = len(node_ids)
        pad_nodes = n_dst_tiles * P - ncnt
        nodes_pad = np.concatenate(
            [node_ids, np.zeros(pad_nodes, np.int64)]) if pad_nodes else node_ids
        valid_row = np.arange(n_dst_tiles * P) < ncnt

        bidcnt = np.zeros((P, n_dst_tiles * 2), np.float32)
        xe_blocks = []
        for t in range(n_dst_tiles):
            Lt = L_list[t]
            nt = nodes_pad[t * P:(t + 1) * P]
            vr = valid_row[t * P:(t + 1) * P]
            lens = np.where(vr, deg[nt], 0)
            mask = slot_ar[Lt] < lens[:, None]           # [P, Lt]
            idt = np.full((P, Lt), n_nodes, np.int64)
            gather_pos = (starts[nt][:, None] + slot_ar[Lt])[mask]
            idt[mask] = src_sorted[gather_pos]
            Et = x_ext[idt]                              # [P, Lt, in_dim]
            Et = Et.reshape(P, Lt // 2, 2, in_dim).transpose(2, 3, 1, 0)
            xe_blocks.append(np.ascontiguousarray(
                Et.reshape(2 * in_dim, (Lt // 2) * P)))
            bidcnt[:, 2 * t] = np.where(vr, x_s_batch[nt], 999.0)
            bidcnt[:, 2 * t + 1] = vr.astype(np.float32)
        xe = np.concatenate(xe_blocks, axis=1).astype(ml_dtypes.bfloat16)

        valid = min(ncnt, n_dst_tiles * P)
        xt_pad = np.zeros((n_xt_dbl * 2 * P, in_dim), np.float32)
        xt_pad[:valid] = x_t[node_ids[:valid]]
        xt_p = _pack_dbl(xt_pad, n_xt_dbl, in_dim)
        in_maps.append({
            "xe": xe, "xt_p": xt_p, "bidcnt": bidcnt,
            "wf2": wf2, "wat2": wat2, "w2b": w2b, "biasb": biasb,
        })

    nc = _build_nc(in_dim, n_dst_tiles, n_xt_dbl, PT_list)
    res = run_bass_kernel_spmd(nc, in_maps, core_ids=list(range(N_CORES)))

    q = np.zeros((P, 2), np.float64)
    for c in range(N_CORES):
        q += res.results[c]["q_out"]
    cnt = np.maximum(q[:, 1], 1.0)
    out = q[:, 0] / cnt
    const = float(fc1_b @ fc3_w[:, 0] + fc3_b[0])
    return (out + const).astype(np.float32)


# revision 11
# speedup vs baseline: 10.0168x; 1.1187x over previous
"""GAT (bipartite GATConv + mean-pool + 2 FC) on 8 Trainium2 NeuronCores.

Strategy: shard destination nodes across the 8 cores (each core owns N/8 dst
nodes and all edges pointing at them) so the segment softmax is fully local to
a core — no collectives needed.  The host stages each core's edge shard as a
dst-major packed copy of x_s rows (pure index manipulation); per core:

  Per dst tile of 128 nodes (one node per partition, degree-sorted so tiles
  have uniform run lengths L): K=128 block-diagonal bf16 matmuls compute
  [h_s | a_s] for two edge slots at a time, landing PSUM results directly in
  the [dst x slot] layout the segment ops need — no DRAM table, no indirect
  DMA.  The segment softmax (max-subtraction skipped: logits bounded, exp
  safe in fp32) and weighted message sum are DVE/ACT ops along each
  partition's run.  A host-prepared one-hot matmul pools relu(out)*W2 into
  per-batch partials; batch counts come from a host bincount.

Pad slots use a host-solved x-vector v_pad with v_pad @ (W*att_src) = -300
per head, so padded edges vanish under exp just like a pad table row.

Host work is limited to index manipulation (edge sorting / padding / packed
layout / one-hot build), weight folding, and the final unsharding reduction
of 8 x [128,1] partials.
"""

import numpy as np
import ml_dtypes

import concourse.bacc as bacc
import concourse.tile as tile
from concourse import mybir
from concourse.bass_utils import run_bass_kernel_spmd

F32 = mybir.dt.float32
BF16 = mybir.dt.bfloat16

N_CORES = 8
P = 128
HEADS = 3
CH = 12
HC = HEADS * CH          # 36
ROW = HC + 4             # matmul out per slot: 36 h | 3 a_s | 1 pad
PAD_A = -300.0           # pad-slot a_s value: exp(0.2*-300) = e^-60 ~ 0
NEG_SLOPE = 0.2
PSB = 512                # fp32 elems per PSUM bank (matmul blocks of 6*80)

_nc_cache = {}


def _build_nc(in_dim, n_dst_tiles, n_xt_dbl, PT_list):
    """Build the SPMD Bass program (identical for all cores)."""
    key = (in_dim, n_dst_tiles, n_xt_dbl, tuple(PT_list))
    if key in _nc_cache:
        return _nc_cache[key]

    PT_max = max(PT_list)
    xe_cols = sum(PT_list) * P

    nc = bacc.Bacc("TRN2", target_bir_lowering=False, debug=False)
    d_xe = nc.dram_tensor("xe", [2 * in_dim, xe_cols], BF16, kind="ExternalInput")
    d_xt = nc.dram_tensor("xt_p", [2 * in_dim, n_xt_dbl * P], BF16, kind="ExternalInput")
    d_oh = nc.dram_tensor("oh", [P, n_dst_tiles * P], BF16, kind="ExternalInput")
    d_wf2 = nc.dram_tensor("wf2", [2 * in_dim, 2 * ROW], BF16, kind="ExternalInput")
    d_wat2 = nc.dram_tensor("wat2", [2 * in_dim, 8], BF16, kind="ExternalInput")
    d_w2 = nc.dram_tensor("w2b", [P, HC], F32, kind="ExternalInput")
    d_bb = nc.dram_tensor("biasb", [P, HC], F32, kind="ExternalInput")
    d_q = nc.dram_tensor("q_out", [P, 1], F32, kind="ExternalOutput")

    with tile.TileContext(nc) as tc:
        with tc.tile_pool(name="const", bufs=1) as cpool, \
             tc.tile_pool(name="xload", bufs=3) as xpool, \
             tc.tile_pool(name="gat", bufs=4) as gpool, \
             tc.tile_pool(name="work", bufs=3) as wpool, \
             tc.tile_pool(name="psA", bufs=2, space="PSUM") as psA, \
             tc.tile_pool(name="psB", bufs=1, space="PSUM") as psB, \
             tc.tile_pool(name="psT", bufs=1, space="PSUM") as psT:

            # ---- constants into SBUF ----
            t_wf2 = cpool.tile([2 * in_dim, 2 * ROW], BF16)
            nc.sync.dma_start(t_wf2[:], d_wf2[:])
            t_wat2 = cpool.tile([2 * in_dim, 8], BF16)
            nc.sync.dma_start(t_wat2[:], d_wat2[:])
            t_w2 = cpool.tile([P, HC], F32)
            nc.sync.dma_start(t_w2[:], d_w2[:])
            t_bb = cpool.tile([P, HC], F32)
            nc.sync.dma_start(t_bb[:], d_bb[:])
            t_oh = cpool.tile([P, n_dst_tiles * P], BF16)
            nc.sync.dma_start(t_oh[:], d_oh[:])
            t_xt = cpool.tile([2 * in_dim, n_xt_dbl * P], BF16)
            nc.sync.dma_start(t_xt[:], d_xt[:])

            t_qacc = cpool.tile([P, 1], F32)
            nc.vector.memset(t_qacc[:], 0.0)

            # ---- a_t per dst double-tile -> resident SBUF ----
            # t_at[:, d*8+(t%2)*4+h] = a_t of node tile t=2d+(t%2), head h
            t_at = cpool.tile([P, n_xt_dbl * 8], F32)
            for d in range(n_xt_dbl):
                ps = psT.tile([P, 8], F32, space="PSUM", tag="psat")
                nc.tensor.matmul(
                    ps[:], lhsT=t_xt[:, d * P:(d + 1) * P], rhs=t_wat2[:],
                    start=True, stop=True)
                nc.scalar.copy(t_at[:, d * 8:(d + 1) * 8], ps[:])

            # ---- main loop over dst tiles ----
            xoff = 0
            for t in range(n_dst_tiles):
                PT = PT_list[t]
                L = 2 * PT
                xe = xpool.tile([2 * in_dim, PT_max * P], BF16, tag="xe")
                nc.sync.dma_start(xe[:, : PT * P], d_xe[:, xoff:xoff + PT * P])
                xoff += PT * P

                # h|a per edge slot via block-diag matmuls (2 slots / matmul);
                # chunks of 18 matmuls fill one 3-bank PSUM tile -> one copy
                g = gpool.tile([P, ((PT_max + 5) // 6) * 6 * 2 * ROW], F32, tag="G")
                for c0 in range(0, PT, 18):
                    nchunk = min(18, PT - c0)
                    nblk = (nchunk + 5) // 6
                    ps = psA.tile([P, 3 * PSB], F32, space="PSUM", tag="psa")
                    for j in range(nchunk):
                        nc.tensor.matmul(
                            ps[:, (j // 6) * PSB + (j % 6) * 2 * ROW:
                               (j // 6) * PSB + (j % 6 + 1) * 2 * ROW],
                            lhsT=xe[:, (c0 + j) * P:(c0 + j + 1) * P],
                            rhs=t_wf2[:], start=True, stop=True)
                    ps3 = ps[:].rearrange("p (b x) -> p b x", x=PSB)
                    gbase = c0 * 2 * ROW
                    g3b = g[:, gbase:gbase + nblk * 6 * 2 * ROW].rearrange(
                        "p (b x) -> p b x", x=6 * 2 * ROW)
                    nc.scalar.copy(g3b, ps3[:, :nblk, : 6 * 2 * ROW])

                g3 = g[:, : L * ROW].rearrange("p (l c) -> p l c", c=ROW)
                atc = (t // 2) * 8 + (t % 2) * 4

                # logits T = a_s + a_t (broadcast over l), leaky = (T*.2) max T
                tT = wpool.tile([P, 2 * PT_max * HEADS], F32, tag="T")
                T3 = tT[:, : L * HEADS].rearrange("p (l h) -> p l h", h=HEADS)
                at_b = t_at[:, atc:atc + HEADS].unsqueeze(1).to_broadcast(
                    (P, L, HEADS))
                nc.vector.tensor_tensor(
                    out=T3[:], in0=g3[:, :, HC:HC + HEADS], in1=at_b,
                    op=mybir.AluOpType.add)
                tLR = wpool.tile([P, 2 * PT_max * HEADS], F32, tag="LR")
                nc.vector.scalar_tensor_tensor(
                    out=tLR[:, : L * HEADS], in0=tT[:, : L * HEADS],
                    scalar=NEG_SLOPE, in1=tT[:, : L * HEADS],
                    op0=mybir.AluOpType.mult, op1=mybir.AluOpType.max)
                tE = wpool.tile([P, 2 * PT_max * HEADS], F32, tag="E")
                nc.scalar.activation(
                    tE[:, : L * HEADS], tLR[:, : L * HEADS],
                    mybir.ActivationFunctionType.Exp)
                E3 = tE[:, : L * HEADS].rearrange("p (l h) -> p l h", h=HEADS)

                # denom + reciprocal
                t_den = wpool.tile([P, HEADS], F32, tag="den")
                nc.vector.tensor_reduce(
                    out=t_den[:], in_=E3.transpose([0, 2, 1]),
                    axis=mybir.AxisListType.X, op=mybir.AluOpType.add)
                t_rec = wpool.tile([P, HEADS], F32, tag="rec")
                nc.vector.reciprocal(t_rec[:], t_den[:])

                # weighted message sum U = sum_l e * h
                tM = wpool.tile([P, 2 * PT_max * HC], F32, tag="M")
                M3 = tM[:, : L * HC].rearrange("p (l j) -> p l j", j=HC)
                e_b = E3.unsqueeze(3).to_broadcast((P, L, HEADS, CH))
                nc.vector.tensor_tensor(
                    out=M3[:], in0=g3[:, :, 0:HC], in1=e_b, op=mybir.AluOpType.mult)
                tU = wpool.tile([P, HC], F32, tag="U")
                nc.vector.tensor_reduce(
                    out=tU[:], in_=M3.transpose([0, 2, 1]),
                    axis=mybir.AxisListType.X, op=mybir.AluOpType.add)

                # out = relu(U / denom + bias)
                tV = wpool.tile([P, HC], F32, tag="V")
                rec_b = t_rec[:].unsqueeze(2).to_broadcast((P, HEADS, CH))
                nc.vector.tensor_tensor(
                    out=tV[:].rearrange("p (h c) -> p h c", c=CH),
                    in0=tU[:].rearrange("p (h c) -> p h c", c=CH),
                    in1=rec_b, op=mybir.AluOpType.mult)
                nc.vector.tensor_tensor(
                    out=tV[:], in0=tV[:], in1=t_bb[:], op=mybir.AluOpType.add)
                nc.scalar.activation(tV[:], tV[:], mybir.ActivationFunctionType.Relu)

                # rv = sum(V * W2) fused into accumulator (bf16 for the
                # pooling matmul: lhsT/rhs dtypes must match)
                tR = wpool.tile([P, HC], F32, tag="R")
                tRV = wpool.tile([P, 1], BF16, tag="RV")
                nc.vector.scalar_tensor_tensor(
                    out=tR[:], in0=tV[:], scalar=1.0, in1=t_w2[:],
                    op0=mybir.AluOpType.mult, op1=mybir.AluOpType.mult,
                    accum_out=tRV[:])

                # pool into batches: q += onehot(bid)^T @ rv  (host one-hot)
                ps_q = psB.tile([P, 1], F32, space="PSUM", tag="q")
                nc.tensor.matmul(
                    ps_q[:], lhsT=t_oh[:, t * P:(t + 1) * P], rhs=tRV[:],
                    start=True, stop=True)
                nc.vector.tensor_tensor(
                    out=t_qacc[:], in0=t_qacc[:], in1=ps_q[:],
                    op=mybir.AluOpType.add)

            nc.sync.dma_start(d_q[:], t_qacc[:])
    nc.finalize()
    _nc_cache[key] = nc
    return nc


def _pack_dbl(x, n_dbl, in_dim):
    """Pack [n_dbl*256, in_dim] node-major features into the K=128
    block-diagonal lhsT layout [2*in_dim, n_dbl*128] (bf16)."""
    a = x.reshape(n_dbl, 2, P, in_dim)
    return np.ascontiguousarray(
        a.transpose(1, 3, 0, 2).reshape(2 * in_dim, n_dbl * P)
    ).astype(ml_dtypes.bfloat16)


def kernel(**inputs):
    x_s = np.asarray(inputs["x_s"], np.float32)
    x_t = np.asarray(inputs["x_t"], np.float32)
    edge_index = np.asarray(inputs["edge_index"])
    x_s_batch = np.asarray(inputs["x_s_batch"]).astype(np.int64)
    W = np.asarray(inputs["W"], np.float32)
    att_src = np.asarray(inputs["att_src"], np.float32)
    att_dst = np.asarray(inputs["att_dst"], np.float32)
    bias = np.asarray(inputs["bias"], np.float32)
    fc1_w = np.asarray(inputs["fc1_w"], np.float32)
    fc1_b = np.asarray(inputs["fc1_b"], np.float32)
    fc3_w = np.asarray(inputs["fc3_w"], np.float32)
    fc3_b = np.asarray(inputs["fc3_b"], np.float32)

    n_nodes, in_dim = x_s.shape
    src = edge_index[0].astype(np.int64)
    dst = edge_index[1].astype(np.int64)

    # ---- host: edge bucketing by destination (layout prep only) ----
    deg = np.bincount(dst, minlength=n_nodes)
    order = np.argsort(-deg, kind="stable")      # nodes by degree desc
    nodes_per_core = (n_nodes + N_CORES - 1) // N_CORES
    n_dst_tiles = (nodes_per_core + P - 1) // P
    L_list = []
    for t in range(n_dst_tiles):
        r0 = t * P * N_CORES
        Lt = max(1, int(deg[order[min(r0, n_nodes - 1)]]))
        L_list.append(Lt + (Lt & 1))             # force even (2 slots/matmul)
    PT_list = [Lt // 2 for Lt in L_list]
    n_xt_dbl = (n_dst_tiles + 1) // 2

    # edges sorted by dst -> per-node contiguous src runs
    e_order = np.argsort(dst, kind="stable")
    src_sorted = src[e_order].astype(np.int64)
    starts = np.searchsorted(dst[e_order], np.arange(n_nodes))

    # fold weights (host weight prep)
    wa_t = np.einsum("khc,hc->kh", W.reshape(in_dim, HEADS, CH), att_dst).astype(np.float32)
    wa_s = np.einsum("khc,hc->kh", W.reshape(in_dim, HEADS, CH), att_src).astype(np.float32)
    wfold = np.zeros((in_dim, ROW), np.float32)
    wfold[:, :HC] = W
    wfold[:, HC:HC + HEADS] = wa_s
    wf2 = np.zeros((2 * in_dim, 2 * ROW), np.float32)
    wf2[:in_dim, :ROW] = wfold
    wf2[in_dim:, ROW:] = wfold
    wf2 = wf2.astype(ml_dtypes.bfloat16)
    wat4 = np.zeros((in_dim, 4), np.float32)
    wat4[:, :HEADS] = wa_t
    wat2 = np.zeros((2 * in_dim, 8), np.float32)
    wat2[:in_dim, :4] = wat4
    wat2[in_dim:, 4:] = wat4
    wat2 = wat2.astype(ml_dtypes.bfloat16)
    w2 = (fc1_w @ fc3_w)[:, 0].astype(np.float32)      # [36]
    w2b = np.tile(w2[None, :], (P, 1))
    biasb = np.tile(bias[None, :], (P, 1))

    # pad-slot x vector: v_pad @ wa_s = PAD_A for every head
    v_pad = wa_s @ np.linalg.solve(
        wa_s.T @ wa_s, np.full((HEADS,), PAD_A, np.float64)).astype(np.float32)
    x_ext = np.vstack([x_s, v_pad[None, :]])           # row n_nodes = pad

    slot_ar = {}
    for Lt in set(L_list):
        slot_ar[Lt] = np.arange(Lt)[None, :]

    in_maps = []
    for c in range(N_CORES):
        node_ids = order[c::N_CORES]             # this core's dst nodes, deg-sorted
        ncnt = len(node_ids)
        pad_nodes = n_dst_tiles * P - ncnt
        nodes_pad = np.concatenate(
            [node_ids, np.zeros(pad_nodes, np.int64)]) if pad_nodes else node_ids
        valid_row = np.arange(n_dst_tiles * P) < ncnt

        oh = np.zeros((P, n_dst_tiles * P), np.float32)
        xe_blocks = []
        for t in range(n_dst_tiles):
            Lt = L_list[t]
            nt = nodes_pad[t * P:(t + 1) * P]
            vr = valid_row[t * P:(t + 1) * P]
            lens = np.where(vr, deg[nt], 0)
            mask = slot_ar[Lt] < lens[:, None]           # [P, Lt]
            idt = np.full((P, Lt), n_nodes, np.int64)
            gather_pos = (starts[nt][:, None] + slot_ar[Lt])[mask]
            idt[mask] = src_sorted[gather_pos]
            Et = x_ext[idt]                              # [P, Lt, in_dim]
            Et = Et.reshape(P, Lt // 2, 2, in_dim).transpose(2, 3, 1, 0)
            xe_blocks.append(np.ascontiguousarray(
                Et.reshape(2 * in_dim, (Lt // 2) * P)))
            rows = np.nonzero(vr)[0]
            oh[rows, t * P + x_s_batch[nt[rows]]] = 1.0
        xe = np.concatenate(xe_blocks, axis=1).astype(ml_dtypes.bfloat16)
        oh = oh.astype(ml_dtypes.bfloat16)

        valid = min(ncnt, n_dst_tiles * P)
        xt_pad = np.zeros((n_xt_dbl * 2 * P, in_dim), np.float32)
        xt_pad[:valid] = x_t[node_ids[:valid]]
        xt_p = _pack_dbl(xt_pad, n_xt_dbl, in_dim)
        in_maps.append({
            "xe": xe, "xt_p": xt_p, "oh": oh,
            "wf2": wf2, "wat2": wat2, "w2b": w2b, "biasb": biasb,
        })

    nc = _build_nc(in_dim, n_dst_tiles, n_xt_dbl, PT_list)
    res = run_bass_kernel_spmd(nc, in_maps, core_ids=list(range(N_CORES)))

    q = np.zeros((P,), np.float64)
    for c in range(N_CORES):
        q += res.results[c]["q_out"][:, 0]
    cnt = np.bincount(x_s_batch, minlength=P).astype(np.float64)
    out = q / np.maximum(cnt, 1.0)
    const = float(fc1_b @ fc3_w[:, 0] + fc3_b[0])
    return (out + const).astype(np.float32)


# revision 18
# speedup vs baseline: 10.4722x; 1.0455x over previous
"""GAT (bipartite GATConv + mean-pool + 2 FC) on 8 Trainium2 NeuronCores.

Strategy: shard destination nodes across the 8 cores (each core owns N/8 dst
nodes and all edges pointing at them) so the segment softmax is fully local to
a core — no collectives needed.  The host stages each core's edge shard as a
dst-major packed copy of x_s rows (pure index manipulation); per core:

  Per dst tile of 128 nodes (one node per partition, degree-sorted so tiles
  have uniform run lengths L): K=128 block-diagonal bf16 matmuls compute
  [h_s | a_s] for two edge slots at a time, landing PSUM results directly in
  the [dst x slot] layout the segment ops need — no DRAM table, no indirect
  DMA.  The segment softmax (max-subtraction skipped: logits bounded, exp
  safe in fp32) and weighted message sum are DVE/ACT ops along each
  partition's run.  A host-prepared one-hot matmul pools relu(out)*W2 into
  per-batch partials; batch counts come from a host bincount.

Pad slots use a host-solved x-vector v_pad with v_pad @ (W*att_src) = -300
per head, so padded edges vanish under exp just like a pad table row.

Host work is limited to index manipulation (edge sorting / padding / packed
layout / one-hot build), weight folding, and the final unsharding reduction
of 8 x [128,1] partials.
"""

import numpy as np
import ml_dtypes

import concourse.bacc as bacc
import concourse.tile as tile
from concourse import mybir
from concourse.bass_utils import run_bass_kernel_spmd

F32 = mybir.dt.float32
BF16 = mybir.dt.bfloat16

N_CORES = 8
P = 128
HEADS = 3
CH = 12
HC = HEADS * CH          # 36
ROW = HC + 4             # matmul out per slot: 36 h | 3 a_s | 1 pad
PAD_A = -300.0           # pad-slot a_s value: exp(0.2*-300) = e^-60 ~ 0
NEG_SLOPE = 0.2
PSB = 512                # fp32 elems per PSUM bank (matmul blocks of 6*80)

_nc_cache = {}


def _build_nc(in_dim, n_dst_tiles, n_xt_dbl, PT_list):
    """Build the SPMD Bass program (identical for all cores)."""
    key = (in_dim, n_dst_tiles, n_xt_dbl, tuple(PT_list))
    if key in _nc_cache:
        return _nc_cache[key]

    PT_max = max(PT_list)
    Lg = ((PT_max + 5) // 6) * 12          # allocated slots per g tile
    xe_cols = sum(PT_list) * P

    nc = bacc.Bacc("TRN2", target_bir_lowering=False, debug=False)
    d_xe = nc.dram_tensor("xe", [2 * in_dim, xe_cols], BF16, kind="ExternalInput")
    d_xt = nc.dram_tensor("xt_p", [2 * in_dim, n_xt_dbl * P], BF16, kind="ExternalInput")
    d_oh = nc.dram_tensor("oh", [P, n_dst_tiles * P], BF16, kind="ExternalInput")
    d_wf2 = nc.dram_tensor("wf2", [2 * in_dim, 2 * ROW], BF16, kind="ExternalInput")
    d_wat2 = nc.dram_tensor("wat2", [2 * in_dim, 8], BF16, kind="ExternalInput")
    d_w2 = nc.dram_tensor("w2b", [P, HC], F32, kind="ExternalInput")
    d_bb = nc.dram_tensor("biasb", [P, HC], F32, kind="ExternalInput")
    d_q = nc.dram_tensor("q_out", [P, 1], F32, kind="ExternalOutput")

    with tile.TileContext(nc) as tc:
        with tc.tile_pool(name="const", bufs=1) as cpool, \
             tc.tile_pool(name="xload", bufs=3) as xpool, \
             tc.tile_pool(name="gat", bufs=4) as gpool, \
             tc.tile_pool(name="work", bufs=3) as wpool, \
             tc.tile_pool(name="psA", bufs=2, space="PSUM") as psA, \
             tc.tile_pool(name="psB", bufs=1, space="PSUM") as psB, \
             tc.tile_pool(name="psT", bufs=1, space="PSUM") as psT:

            # ---- constants into SBUF ----
            t_wf2 = cpool.tile([2 * in_dim, 2 * ROW], BF16)
            nc.sync.dma_start(t_wf2[:], d_wf2[:])
            t_wat2 = cpool.tile([2 * in_dim, 8], BF16)
            nc.sync.dma_start(t_wat2[:], d_wat2[:])
            t_w2 = cpool.tile([P, HC], F32)
            nc.sync.dma_start(t_w2[:], d_w2[:])
            t_bb = cpool.tile([P, HC], F32)
            nc.sync.dma_start(t_bb[:], d_bb[:])
            t_oh = cpool.tile([P, n_dst_tiles * P], BF16)
            nc.sync.dma_start(t_oh[:], d_oh[:])
            t_xt = cpool.tile([2 * in_dim, n_xt_dbl * P], BF16)
            nc.sync.dma_start(t_xt[:], d_xt[:])

            t_qacc = cpool.tile([P, 1], F32)
            nc.vector.memset(t_qacc[:], 0.0)

            # ---- a_t per dst double-tile -> resident SBUF ----
            # t_at[:, d*8+(t%2)*4+h] = a_t of node tile t=2d+(t%2), head h
            t_at = cpool.tile([P, n_xt_dbl * 8], F32)
            for d in range(n_xt_dbl):
                ps = psT.tile([P, 8], F32, space="PSUM", tag="psat")
                nc.tensor.matmul(
                    ps[:], lhsT=t_xt[:, d * P:(d + 1) * P], rhs=t_wat2[:],
                    start=True, stop=True)
                nc.scalar.copy(t_at[:, d * 8:(d + 1) * 8], ps[:])

            # ---- main loop over dst tiles ----
            xoff = 0
            for t in range(n_dst_tiles):
                PT = PT_list[t]
                L = 2 * PT
                xe = xpool.tile([2 * in_dim, PT_max * P], BF16, tag="xe")
                nc.sync.dma_start(xe[:, : PT * P], d_xe[:, xoff:xoff + PT * P])
                xoff += PT * P

                # h|a per edge slot via block-diag matmuls (2 slots / matmul,
                # rhs columns (c, half)-interleaved); chunks of 18 matmuls
                # fill one 3-bank PSUM tile -> one transposing copy into
                # channel-major g [p, c, l] so every phase-B op reads
                # contiguous along l
                g = gpool.tile([P, ROW * Lg], F32, tag="G")
                gT = g[:].rearrange("p (c l) -> p c l", l=Lg)
                for c0 in range(0, PT, 18):
                    nchunk = min(18, PT - c0)
                    nblk = (nchunk + 5) // 6
                    ps = psA.tile([P, 3 * PSB], F32, space="PSUM", tag="psa")
                    for j in range(nchunk):
                        nc.tensor.matmul(
                            ps[:, (j // 6) * PSB + (j % 6) * 2 * ROW:
                               (j // 6) * PSB + (j % 6 + 1) * 2 * ROW],
                            lhsT=xe[:, (c0 + j) * P:(c0 + j + 1) * P],
                            rhs=t_wf2[:], start=True, stop=True)
                    ps5 = ps[:].rearrange("p (jb x) -> p jb x", x=PSB)
                    for jb in range(nblk):
                        ps4 = ps5[:, jb, : 6 * 2 * ROW].rearrange(
                            "p (jj c half) -> p c jj half", c=ROW, half=2)
                        l0 = c0 * 2 + jb * 12
                        g4 = gT[:, :, l0:l0 + 12].rearrange(
                            "p c (jj half) -> p c jj half", half=2)
                        nc.scalar.copy(g4, ps4)

                atc = (t // 2) * 8 + (t % 2) * 4

                # logits T = a_s + a_t (h-major, contiguous), leaky fused
                tT = wpool.tile([P, 2 * PT_max * HEADS], F32, tag="T")
                T3 = tT[:, : L * HEADS].rearrange("p (h l) -> p h l", h=HEADS)
                at_b = t_at[:, atc:atc + HEADS].unsqueeze(2).to_broadcast(
                    (P, HEADS, L))
                nc.vector.tensor_tensor(
                    out=T3[:], in0=gT[:, HC:HC + HEADS, :L], in1=at_b,
                    op=mybir.AluOpType.add)
                tLR = wpool.tile([P, 2 * PT_max * HEADS], F32, tag="LR")
                nc.vector.scalar_tensor_tensor(
                    out=tLR[:, : L * HEADS], in0=tT[:, : L * HEADS],
                    scalar=NEG_SLOPE, in1=tT[:, : L * HEADS],
                    op0=mybir.AluOpType.mult, op1=mybir.AluOpType.max)
                tE = wpool.tile([P, 2 * PT_max * HEADS], F32, tag="E")
                nc.scalar.activation(
                    tE[:, : L * HEADS], tLR[:, : L * HEADS],
                    mybir.ActivationFunctionType.Exp)
                E3h = tE[:, : L * HEADS].rearrange("p (h l) -> p h l", h=HEADS)

                # denom + reciprocal
                t_den = wpool.tile([P, HEADS], F32, tag="den")
                nc.vector.tensor_reduce(
                    out=t_den[:], in_=E3h, axis=mybir.AxisListType.X,
                    op=mybir.AluOpType.add)
                t_rec = wpool.tile([P, HEADS], F32, tag="rec")
                nc.vector.reciprocal(t_rec[:], t_den[:])

                # weighted message sum U = sum_l e * h  (channel-major M)
                tM = wpool.tile([P, 2 * PT_max * HC], F32, tag="M")
                M3 = tM[:, : L * HC].rearrange("p (j l) -> p j l", l=L)
                e_b = E3h.unsqueeze(2).to_broadcast((P, HEADS, CH, L))
                nc.vector.tensor_tensor(
                    out=M3[:], in0=gT[:, 0:HC, :L], in1=e_b,
                    op=mybir.AluOpType.mult)
                tU = wpool.tile([P, HC], F32, tag="U")
                nc.vector.tensor_reduce(
                    out=tU[:], in_=M3, axis=mybir.AxisListType.X,
                    op=mybir.AluOpType.add)

                # out = relu(U / denom + bias)  (small V ops on gpsimd)
                tV = wpool.tile([P, HC], F32, tag="V")
                rec_b = t_rec[:].unsqueeze(2).to_broadcast((P, HEADS, CH))
                nc.gpsimd.tensor_tensor(
                    out=tV[:].rearrange("p (h c) -> p h c", c=CH),
                    in0=tU[:].rearrange("p (h c) -> p h c", c=CH),
                    in1=rec_b, op=mybir.AluOpType.mult)
                nc.gpsimd.tensor_tensor(
                    out=tV[:], in0=tV[:], in1=t_bb[:], op=mybir.AluOpType.add)
                nc.scalar.activation(tV[:], tV[:], mybir.ActivationFunctionType.Relu)

                # rv = sum(V * W2) fused into accumulator (bf16 for the
                # pooling matmul: lhsT/rhs dtypes must match)
                tR = wpool.tile([P, HC], F32, tag="R")
                tRV = wpool.tile([P, 1], BF16, tag="RV")
                nc.vector.scalar_tensor_tensor(
                    out=tR[:], in0=tV[:], scalar=1.0, in1=t_w2[:],
                    op0=mybir.AluOpType.mult, op1=mybir.AluOpType.mult,
                    accum_out=tRV[:])

                # pool into batches: q += onehot(bid)^T @ rv  (host one-hot)
                ps_q = psB.tile([P, 1], F32, space="PSUM", tag="q")
                nc.tensor.matmul(
                    ps_q[:], lhsT=t_oh[:, t * P:(t + 1) * P], rhs=tRV[:],
                    start=True, stop=True)
                nc.vector.tensor_tensor(
                    out=t_qacc[:], in0=t_qacc[:], in1=ps_q[:],
                    op=mybir.AluOpType.add)

            nc.sync.dma_start(d_q[:], t_qacc[:])
    nc.finalize()
    _nc_cache[key] = nc
    return nc


def _pack_dbl(x, n_dbl, in_dim):
    """Pack [n_dbl*256, in_dim] node-major features into the K=128
    block-diagonal lhsT layout [2*in_dim, n_dbl*128] (bf16)."""
    a = x.reshape(n_dbl, 2, P, in_dim)
    return np.ascontiguousarray(
        a.transpose(1, 3, 0, 2).reshape(2 * in_dim, n_dbl * P)
    ).astype(ml_dtypes.bfloat16)


def kernel(**inputs):
    x_s = np.asarray(inputs["x_s"], np.float32)
    x_t = np.asarray(inputs["x_t"], np.float32)
    edge_index = np.asarray(inputs["edge_index"])
    x_s_batch = np.asarray(inputs["x_s_batch"]).astype(np.int64)
    W = np.asarray(inputs["W"], np.float32)
    att_src = np.asarray(inputs["att_src"], np.float32)
    att_dst = np.asarray(inputs["att_dst"], np.float32)
    bias = np.asarray(inputs["bias"], np.float32)
    fc1_w = np.asarray(inputs["fc1_w"], np.float32)
    fc1_b = np.asarray(inputs["fc1_b"], np.float32)
    fc3_w = np.asarray(inputs["fc3_w"], np.float32)
    fc3_b = np.asarray(inputs["fc3_b"], np.float32)

    n_nodes, in_dim = x_s.shape
    src = edge_index[0].astype(np.int64)
    dst = edge_index[1].astype(np.int64)

    # ---- host: edge bucketing by destination (layout prep only) ----
    deg = np.bincount(dst, minlength=n_nodes)
    order = np.argsort(-deg, kind="stable")      # nodes by degree desc
    nodes_per_core = (n_nodes + N_CORES - 1) // N_CORES
    n_dst_tiles = (nodes_per_core + P - 1) // P
    L_list = []
    for t in range(n_dst_tiles):
        r0 = t * P * N_CORES
        Lt = max(1, int(deg[order[min(r0, n_nodes - 1)]]))
        L_list.append(Lt + (Lt & 1))             # force even (2 slots/matmul)
    PT_list = [Lt // 2 for Lt in L_list]
    n_xt_dbl = (n_dst_tiles + 1) // 2

    # edges sorted by dst -> per-node contiguous src runs
    e_order = np.argsort(dst, kind="stable")
    src_sorted = src[e_order].astype(np.int64)
    starts = np.searchsorted(dst[e_order], np.arange(n_nodes))

    # fold weights (host weight prep)
    wa_t = np.einsum("khc,hc->kh", W.reshape(in_dim, HEADS, CH), att_dst).astype(np.float32)
    wa_s = np.einsum("khc,hc->kh", W.reshape(in_dim, HEADS, CH), att_src).astype(np.float32)
    wfold = np.zeros((in_dim, ROW), np.float32)
    wfold[:, :HC] = W
    wfold[:, HC:HC + HEADS] = wa_s
    wf2 = np.zeros((2 * in_dim, 2 * ROW), np.float32)
    wf2[:in_dim, :ROW] = wfold
    wf2[in_dim:, ROW:] = wfold
    # interleave output columns to (c, half) so PSUM->g copies land
    # channel-major: new col c*2+half = old col half*ROW+c
    wf2 = np.ascontiguousarray(
        wf2.reshape(2 * in_dim, 2, ROW).transpose(0, 2, 1).reshape(
            2 * in_dim, 2 * ROW)).astype(ml_dtypes.bfloat16)
    wat4 = np.zeros((in_dim, 4), np.float32)
    wat4[:, :HEADS] = wa_t
    wat2 = np.zeros((2 * in_dim, 8), np.float32)
    wat2[:in_dim, :4] = wat4
    wat2[in_dim:, 4:] = wat4
    wat2 = wat2.astype(ml_dtypes.bfloat16)
    w2 = (fc1_w @ fc3_w)[:, 0].astype(np.float32)      # [36]
    w2b = np.tile(w2[None, :], (P, 1))
    biasb = np.tile(bias[None, :], (P, 1))

    # pad-slot x vector: v_pad @ wa_s = PAD_A for every head
    v_pad = wa_s @ np.linalg.solve(
        wa_s.T @ wa_s, np.full((HEADS,), PAD_A, np.float64)).astype(np.float32)
    x_ext = np.vstack([x_s, v_pad[None, :]])           # row n_nodes = pad

    slot_ar = {}
    for Lt in set(L_list):
        slot_ar[Lt] = np.arange(Lt)[None, :]

    in_maps = []
    for c in range(N_CORES):
        node_ids = order[c::N_CORES]             # this core's dst nodes, deg-sorted
        ncnt = len(node_ids)
        pad_nodes = n_dst_tiles * P - ncnt
        nodes_pad = np.concatenate(
            [node_ids, np.zeros(pad_nodes, np.int64)]) if pad_nodes else node_ids
        valid_row = np.arange(n_dst_tiles * P) < ncnt

        oh = np.zeros((P, n_dst_tiles * P), np.float32)
        xe_blocks = []
        for t in range(n_dst_tiles):
            Lt = L_list[t]
            nt = nodes_pad[t * P:(t + 1) * P]
            vr = valid_row[t * P:(t + 1) * P]
            lens = np.where(vr, deg[nt], 0)
            mask = slot_ar[Lt] < lens[:, None]           # [P, Lt]
            idt = np.full((P, Lt), n_nodes, np.int64)
            gather_pos = (starts[nt][:, None] + slot_ar[Lt])[mask]
            idt[mask] = src_sorted[gather_pos]
            Et = x_ext[idt]                              # [P, Lt, in_dim]
            Et = Et.reshape(P, Lt // 2, 2, in_dim).transpose(2, 3, 1, 0)
            xe_blocks.append(np.ascontiguousarray(
                Et.reshape(2 * in_dim, (Lt // 2) * P)))
            rows = np.nonzero(vr)[0]
            oh[rows, t * P + x_s_batch[nt[rows]]] = 1.0
        xe = np.concatenate(xe_blocks, axis=1).astype(ml_dtypes.bfloat16)
        oh = oh.astype(ml_dtypes.bfloat16)

        valid = min(ncnt, n_dst_tiles * P)
        xt_pad = np.zeros((n_xt_dbl * 2 * P, in_dim), np.float32)
        xt_pad[:valid] = x_t[node_ids[:valid]]
        xt_p = _pack_dbl(xt_pad, n_xt_dbl, in_dim)
        in_maps.append({
            "xe": xe, "xt_p": xt_p, "oh": oh,
            "wf2": wf2, "wat2": wat2, "w2b": w2b, "biasb": biasb,
        })

    nc = _build_nc(in_dim, n_dst_tiles, n_xt_dbl, PT_list)
    res = run_bass_kernel_spmd(nc, in_maps, core_ids=list(range(N_CORES)))

    q = np.zeros((P,), np.float64)
    for c in range(N_CORES):
        q += res.results[c]["q_out"][:, 0]
    cnt = np.bincount(x_s_batch, minlength=P).astype(np.float64)
    out = q / np.maximum(cnt, 1.0)
    const = float(fc1_b @ fc3_w[:, 0] + fc3_b[0])
    return (out + const).astype(np.float32)
